# revision 1
# baseline (speedup 1.0000x reference)
"""Trainium2 Bass kernel for nn_DecoderLayer (dense transformer decoder layer).

Distribution over 8 NeuronCores: core c = 2*b + r handles batch b (of 4) with
tensor-parallel rank r (of 2).
  - QKV + attention: rank r computes heads [8r, 8r+8) for ALL 2048 tokens of
    its batch. Q/K are produced head-transposed ([d_k, T]); V natural, with a
    ones-column appended per head so the PV matmul also yields the softmax
    denominator. Causality is exploited block-wise (upper blocks skipped,
    diagonal blocks masked after exp).
  - Two pairwise AllGathers (2 MB each per rank) re-shard the attention
    context from head-split to token-split, overlapped with compute.
  - wo projection, LN1, FFN, LN2 run fully local on the rank's own
    1024-token half. Rank-specific columns of the AllGather outputs are read
    via a register-driven dynamic DMA offset so one SPMD program serves both
    ranks.
All compute is fp32. The host only reshapes/transposes (data movement).
"""

import contextlib

import numpy as np

import concourse.bass as bass
import concourse.bacc as bacc
import concourse.mybir as mybir
import concourse.tile as tile
from concourse.bass_utils import run_bass_kernel_spmd

F32 = mybir.dt.float32
F32R = mybir.dt.float32r
U32 = mybir.dt.uint32
AF = mybir.ActivationFunctionType
X_AXIS = mybir.AxisListType.X

N_CORES = 8
FULL_CFG = dict(T=2048, D=1024, H=16, DK=64, FF=4096, B=4)


def derive(cfg):
    T, D, H, DK, FF, B = (cfg[k] for k in ("T", "D", "H", "DK", "FF", "B"))
    assert DK == 64 and H % 2 == 0 and T % 512 == 0 and D % 128 == 0
    HPC = H // 2
    DS = HPC * DK
    c = dict(cfg)
    c.update(
        HPC=HPC, DS=DS,
        CH=T // 4, KTN=T // 128, DT=D // 128, DSP=DS // 128,
        FT=FF // 128, HALF=T // 2,
    )
    c["TCK"] = c["CH"] // 128
    c["MT"] = c["CH"] // 128
    c["WN"] = min(512, D)
    c["NWC"] = D // c["WN"]
    assert c["CH"] <= 512
    return c


def build_nc(cfg, amp_reps=1, sim_mode=False, mm_dt=F32R):
    c = derive(cfg)
    T, D, FF = c["T"], c["D"], c["FF"]
    CH, KTN, DT, DSP, FT, MT = c["CH"], c["KTN"], c["DT"], c["DSP"], c["FT"], c["MT"]
    TCK, WN, NWC, DS, HALF, HPC = (
        c["TCK"], c["WN"], c["NWC"], c["DS"], c["HALF"], c["HPC"])
    scale = float(1.0 / np.sqrt(c["DK"]))

    nc = bacc.Bacc("TRN2", target_bir_lowering=False, num_devices=N_CORES)

    xT_d = nc.dram_tensor("xT", [D, T], mm_dt, kind="ExternalInput")
    xown_d = nc.dram_tensor("x_own", [HALF, D], F32, kind="ExternalInput")
    wqT_d = nc.dram_tensor("wqT", [D, DS], mm_dt, kind="ExternalInput")
    wkT_d = nc.dram_tensor("wkT", [D, DS], mm_dt, kind="ExternalInput")
    wvT_d = nc.dram_tensor("wvT", [D, DS], mm_dt, kind="ExternalInput")
    bq8_d = nc.dram_tensor("bq8", [DSP, 128, 1], F32, kind="ExternalInput")
    bk_d = nc.dram_tensor("bk", [DSP, 128, 1], F32, kind="ExternalInput")
    bvr_d = nc.dram_tensor("bv_rep", [128, DS], F32, kind="ExternalInput")
    woT_d = nc.dram_tensor("woT", [D, D], mm_dt, kind="ExternalInput")
    bor_d = nc.dram_tensor("bo_rep", [128, D], F32, kind="ExternalInput")
    w1p_d = nc.dram_tensor("w1T_packed", [128, DT * FF], mm_dt, kind="ExternalInput")
    b1c_d = nc.dram_tensor("b1c", [FT, 128, 1], F32, kind="ExternalInput")
    w2T_d = nc.dram_tensor("w2T", [FF, D], mm_dt, kind="ExternalInput")
    b2r_d = nc.dram_tensor("b2_rep", [128, D], F32, kind="ExternalInput")
    ident_d = nc.dram_tensor("ident", [128, 128], F32, kind="ExternalInput")
    masks_d = nc.dram_tensor("masks", [TCK, 128, CH], mm_dt, kind="ExternalInput")
    ones_d = nc.dram_tensor("ones_bc", [1, 64], mm_dt, kind="ExternalInput")
    coloff_d = nc.dram_tensor("coloff", [1, 1], U32, kind="ExternalInput")
    out_d = nc.dram_tensor("out_own", [HALF, D], F32, kind="ExternalOutput")

    RG = [[0, 1], [2, 3], [4, 5], [6, 7]]

    with tile.TileContext(nc) as tc:
        with contextlib.ExitStack() as es:
            p_const = es.enter_context(tc.tile_pool(name="const", bufs=1))
            p_dram = es.enter_context(tc.tile_pool(name="dram", bufs=1, space="DRAM"))

            ident = p_const.tile([128, 128], F32, tag="ident", name="ident")
            nc.sync.dma_start(ident[:], ident_d[:])
            ones_bc = p_const.tile([1, 64], mm_dt, tag="ones_bc", name="ones_bc")
            nc.sync.dma_start(ones_bc[:], ones_d[:])

            ag1_in = p_dram.tile([DS, 2 * CH], mm_dt, tag="ag1i", name="ag1i")
            ag1_out = p_dram.tile([2 * DS, 2 * CH], mm_dt, tag="ag1o", name="ag1o")
            ag2_in = p_dram.tile([DS, 2 * CH], mm_dt, tag="ag2i", name="ag2i")
            ag2_out = p_dram.tile([2 * DS, 2 * CH], mm_dt, tag="ag2o", name="ag2o")

            offsb = p_const.tile([1, 1], U32, tag="offsb", name="offsb")
            nc.sync.dma_start(offsb[:], coloff_d[:])
            roff = nc.sync.alloc_register("roff")
            nc.sync.reg_load(roff, offsb[0:1, 0:1])
            off = nc.sync.snap(roff, min_val=0, max_val=CH)

            def emit_allgather(ag_i, ag_o):
                if sim_mode:
                    nc.scalar.dma_start(ag_o[0:DS, :], ag_i[:])
                    nc.scalar.dma_start(ag_o[DS:2 * DS, :], ag_i[:])
                else:
                    nc.gpsimd.collective_compute(
                        "AllGather", mybir.AluOpType.bypass,
                        replica_groups=RG,
                        ins=[ag_i.opt()], outs=[ag_o.opt()],
                    )

            def emit_layer():
                # ======== Phases 1+2: QKV projections and attention ========
                p_ctx_cm = tc.tile_pool(name="ctx3", bufs=1)
                p_ctx = p_ctx_cm.__enter__()
                with (
                    tc.tile_pool(name="qt", bufs=1) as p_qt,
                    tc.tile_pool(name="kt", bufs=1) as p_kt,
                    tc.tile_pool(name="vaug", bufs=1) as p_va,
                ):
                    QT = [p_qt.tile([128, T], mm_dt, tag=f"q{p}", name=f"q{p}")
                          for p in range(DSP)]
                    KT = [p_kt.tile([128, T], mm_dt, tag=f"k{p}", name=f"k{p}")
                          for p in range(DSP)]
                    VA = [p_va.tile([128, HPC * 65], mm_dt, tag=f"v{i}", name=f"v{i}")
                          for i in range(KTN)]

                    # ---- Phase 1: QKV ----
                    with (
                        tc.tile_pool(name="xT", bufs=1) as p_xT,
                        tc.tile_pool(name="wst", bufs=4) as p_wst,
                        tc.tile_pool(name="wv", bufs=1) as p_wv,
                        tc.tile_pool(name="bias1", bufs=1) as p_b1,
                        tc.tile_pool(name="ps_mm1", bufs=1, space="PSUM") as ps_mm1,
                    ):
                        XT = [p_xT.tile([128, T], mm_dt, tag=f"x{k}", name=f"x{k}")
                              for k in range(DT)]
                        bvr = p_b1.tile([128, DS], F32, tag="bvr", name="bvr")
                        NQ = T // CH
                        xT_loaded = False
                        for (w_d, b_d, dst, sc, bt) in (
                            (wqT_d, bq8_d, QT, scale, "bq"),
                            (wkT_d, bk_d, KT, 1.0, "bk"),
                        ):
                            BB = [p_b1.tile([128, 1], F32, tag=f"{bt}{p}",
                                            name=f"{bt}{p}")
                                  for p in range(DSP)]
                            for p in range(DSP):
                                nc.sync.dma_start(BB[p][:], b_d[p])
                            for ph in range(0, DSP, 2):
                                plist = [q for q in (ph, ph + 1) if q < DSP]
                                PSN = {
                                    (p, n): ps_mm1.tile(
                                        [128, CH], F32,
                                        tag=f"mm1_{(p % 2) * NQ + n}",
                                        name="psn")
                                    for p in plist for n in range(NQ)
                                }
                                for k in range(DT):
                                    wbs = {}
                                    for p in plist:
                                        wblk = p_wst.tile(
                                            [128, 128], mm_dt,
                                            tag=f"wb{p % 2}", name="wblk")
                                        nc.sync.dma_start(
                                            wblk[:],
                                            w_d[128 * k:128 * (k + 1),
                                                128 * p:128 * (p + 1)])
                                        wbs[p] = wblk
                                    if not xT_loaded:
                                        for n in range(NQ):
                                            nc.sync.dma_start(
                                                XT[k][:, CH * n:CH * (n + 1)],
                                                xT_d[128 * k:128 * (k + 1),
                                                     CH * n:CH * (n + 1)])
                                    for p in plist:
                                        for n in range(NQ):
                                            nc.tensor.matmul(
                                                PSN[p, n][:], wbs[p][:],
                                                XT[k][:, CH * n:CH * (n + 1)],
                                                start=(k == 0),
                                                stop=(k == DT - 1),
                                            )
                                xT_loaded = True
                                for p in plist:
                                    for n in range(NQ):
                                        nc.scalar.activation(
                                            dst[p][:, CH * n:CH * (n + 1)],
                                            PSN[p, n][:],
                                            AF.Identity, bias=BB[p][:], scale=sc,
                                        )
                        nc.sync.dma_start(bvr[:], bvr_d[:])
                        WV = [p_wv.tile([128, DS], mm_dt, tag=f"wv{k}",
                                        name=f"wv{k}")
                              for k in range(DT)]
                        for k in range(DT):
                            nc.sync.dma_start(WV[k][:],
                                              wvT_d[128 * k:128 * (k + 1), :])
                        for i in range(KTN):
                            ps = ps_mm1.tile([128, DS], F32, tag=f"mm1_{i % 8}",
                                             name="psv")
                            for k in range(DT):
                                nc.tensor.matmul(
                                    ps[:], XT[k][:, 128 * i:128 * (i + 1)],
                                    WV[k][:],
                                    start=(k == 0), stop=(k == DT - 1),
                                )
                            va3 = VA[i][:].rearrange("p (h e) -> p h e", e=65)
                            nc.vector.tensor_add(
                                va3[:, :, 0:64],
                                ps[:].rearrange("p (h e) -> p h e", e=64),
                                bvr[:].rearrange("p (h e) -> p h e", e=64),
                            )
                            for h in range(HPC):
                                nc.vector.memset(
                                    VA[i][:, 65 * h + 64:65 * h + 65].bitcast(F32),
                                    1.0)

                    # ---- Phase 2: attention ----
                    with (
                        tc.tile_pool(name="ctxT", bufs=1) as p_ctxT,
                        tc.tile_pool(name="mask", bufs=1) as p_mask,
                        tc.tile_pool(name="exp", bufs=3) as p_exp,
                        tc.tile_pool(name="sm", bufs=2) as p_sm,
                        tc.tile_pool(name="ps_s", bufs=2, space="PSUM") as ps_s,
                        tc.tile_pool(name="ps_pv", bufs=2, space="PSUM") as ps_pv,
                    ):
                        CTX = [p_ctxT.tile([128, T], mm_dt, tag=f"c{p}",
                                           name=f"c{p}")
                               for p in range(DSP)]
                        MSK = [p_mask.tile([128, CH], mm_dt, tag=f"m{m}",
                                           name=f"m{m}")
                               for m in range(TCK)]
                        for m in range(TCK):
                            nc.sync.dma_start(MSK[m][:], masks_d[m])

                        for qi, qc in enumerate([0, 2, 1, 3]):
                            nkt = (qc + 1) * TCK
                            for p in range(DSP):
                                pvA = ps_pv.tile([65, CH], F32, tag="pvA",
                                                 name="pvA")
                                pvB = ps_pv.tile([65, CH], F32, tag="pvB",
                                                 name="pvB")
                                for kt in range(nkt):
                                    m = kt - qc * TCK
                                    c0 = 128 * m if m > 0 else 0
                                    cs = slice(c0, CH)
                                    sA = ps_s.tile([128, CH], F32, tag="sA",
                                                   name="sA")
                                    sB = ps_s.tile([128, CH], F32, tag="sB",
                                                   name="sB")
                                    nc.tensor.matmul(
                                        sA[:, cs],
                                        KT[p][0:64, 128 * kt:128 * (kt + 1)],
                                        QT[p][0:64, CH * qc + c0:CH * (qc + 1)],
                                        start=True, stop=True,
                                    )
                                    nc.tensor.matmul(
                                        sB[:, cs],
                                        KT[p][64:128, 128 * kt:128 * (kt + 1)],
                                        QT[p][64:128, CH * qc + c0:CH * (qc + 1)],
                                        start=True, stop=True,
                                        tile_position=(64, 0),
                                    )
                                    eA = p_exp.tile([128, CH], mm_dt, tag="eA",
                                                    name="eA")
                                    eB = p_exp.tile([128, CH], mm_dt, tag="eB",
                                                    name="eB")
                                    nc.scalar.activation(eA[:, cs], sA[:, cs],
                                                         AF.Exp)
                                    nc.scalar.activation(eB[:, cs], sB[:, cs],
                                                         AF.Exp)
                                    if m >= 0:
                                        nc.vector.tensor_mul(eA[:, cs], eA[:, cs],
                                                             MSK[m][:, cs])
                                        nc.vector.tensor_mul(eB[:, cs], eB[:, cs],
                                                             MSK[m][:, cs])
                                    nc.tensor.matmul(
                                        pvA[:, cs],
                                        VA[kt][:, 130 * p:130 * p + 65],
                                        eA[:, cs],
                                        start=(kt == 0), stop=(kt == nkt - 1),
                                    )
                                    nc.tensor.matmul(
                                        pvB[:, cs],
                                        VA[kt][:, 130 * p + 65:130 * p + 130],
                                        eB[:, cs],
                                        start=(kt == 0), stop=(kt == nkt - 1),
                                    )
                                for hi, pv in ((0, pvA), (1, pvB)):
                                    rec = p_sm.tile([1, CH], mm_dt, tag="rec",
                                                    name="rec")
                                    with nc.allow_low_precision(
                                            reason="f32r recip -> f32r matmul"):
                                        nc.vector.reciprocal(rec[:],
                                                             pv[64:65, :])
                                    bc = ps_s.tile([128, CH], F32, tag="sA",
                                                   name="bc")
                                    nc.tensor.matmul(bc[0:64, :], ones_bc[:],
                                                     rec[:],
                                                     start=True, stop=True)
                                    bcs = p_exp.tile([128, CH], F32, tag="bcs",
                                                     name="bcs")
                                    nc.vector.tensor_copy(bcs[0:64, :],
                                                          bc[0:64, :])
                                    nc.vector.tensor_mul(
                                        CTX[p][64 * hi:64 * (hi + 1),
                                               CH * qc:CH * (qc + 1)],
                                        pv[0:64, :], bcs[0:64, :],
                                    )
                            if qi == 1:
                                for p in range(DSP):
                                    nc.scalar.dma_start(
                                        ag1_in[128 * p:128 * (p + 1), 0:CH],
                                        CTX[p][:, 0:CH])
                                    nc.scalar.dma_start(
                                        ag1_in[128 * p:128 * (p + 1), CH:2 * CH],
                                        CTX[p][:, 2 * CH:3 * CH])
                                emit_allgather(ag1_in, ag1_out)
                        # chunk-A context prefetch: only waits on AG#1
                        CTX3A = [p_ctx.tile([128, CH], mm_dt, tag=f"ctx{k}",
                                            name=f"ctx{k}")
                                 for k in range(DT)]
                        for k in range(DT):
                            nc.sync.dma_start(
                                CTX3A[k][:],
                                ag1_out[128 * k:128 * (k + 1)][:,
                                                               bass.ds(off, CH)])
                        # AG#2 staging (waits end of attention)
                        for p in range(DSP):
                            nc.scalar.dma_start(
                                ag2_in[128 * p:128 * (p + 1), 0:CH],
                                CTX[p][:, CH:2 * CH])
                            nc.scalar.dma_start(
                                ag2_in[128 * p:128 * (p + 1), CH:2 * CH],
                                CTX[p][:, 3 * CH:4 * CH])
                        emit_allgather(ag2_in, ag2_out)

                # ======== Phase 3: wo + LN1 + FFN + LN2, per token chunk ========
                with (
                    tc.tile_pool(name="rep", bufs=1) as p_rep,
                    tc.tile_pool(name="xo", bufs=1) as p_xo,
                    tc.tile_pool(name="hh", bufs=1) as p_h,
                    tc.tile_pool(name="hT", bufs=1) as p_hT,
                    tc.tile_pool(name="uT", bufs=1) as p_uT,
                    tc.tile_pool(name="acc", bufs=1) as p_acc,
                    tc.tile_pool(name="wk", bufs=2) as p_work,
                    tc.tile_pool(name="wos", bufs=4) as p_wos,
                    tc.tile_pool(name="w1s", bufs=3) as p_w1s,
                    tc.tile_pool(name="w2s", bufs=6) as p_w2s,
                    tc.tile_pool(name="st", bufs=4) as p_st,
                    tc.tile_pool(name="ps_mm", bufs=2, space="PSUM") as ps_mm,
                    tc.tile_pool(name="ps_tp", bufs=2, space="PSUM") as ps_tp,
                    tc.tile_pool(name="ps_f2", bufs=1, space="PSUM") as ps_f2,
                ):
                    bor = p_rep.tile([128, D], F32, tag="bor", name="bor")
                    nc.scalar.dma_start(bor[:], bor_d[:])
                    b2r = p_rep.tile([128, D], F32, tag="b2r", name="b2r")
                    nc.scalar.dma_start(b2r[:], b2r_d[:])
                    B1C = [p_rep.tile([128, 1], F32, tag=f"b1c{i}",
                                      name=f"b1c{i}")
                           for i in range(FT)]
                    for i in range(FT):
                        nc.gpsimd.dma_start(B1C[i][:], b1c_d[i])

                    def layer_norm(x_in, x_out, scratch):
                        s = p_st.tile([128, 1], F32, tag="s0", name="s0")
                        nc.vector.tensor_reduce(
                            s[:], x_in[:], axis=X_AXIS,
                            op=mybir.AluOpType.add, negate=True)
                        nm = p_st.tile([128, 1], F32, tag="s1", name="s1")
                        nc.scalar.mul(nm[:], s[:], 1.0 / D)
                        xc = p_work.tile([128, D], F32, tag="xc", name="xc")
                        nc.vector.tensor_scalar_add(xc[:], x_in[:], nm[:])
                        sq = p_st.tile([128, 1], F32, tag="s2", name="s2")
                        nc.scalar.activation(scratch[:], xc[:], AF.Square,
                                             accum_out=sq[:])
                        sd = p_st.tile([128, 1], F32, tag="s3", name="s3")
                        nc.scalar.activation(sd[:], sq[:], AF.Sqrt,
                                             scale=1.0 / (D - 1))
                        sde = p_st.tile([128, 1], F32, tag="s4", name="s4")
                        nc.vector.tensor_scalar_add(sde[:], sd[:], 1e-6)
                        rs = p_st.tile([128, 1], F32, tag="s5", name="s5")
                        nc.vector.reciprocal(rs[:], sde[:])
                        nc.vector.tensor_scalar_mul(x_out[:], xc[:], rs[:])

                    for ci, ag_out in ((0, ag1_out), (1, ag2_out)):
                        if ci == 0:
                            CTX3 = CTX3A
                        else:
                            CTX3 = [p_ctx.tile([128, CH], mm_dt, tag=f"ctx{k}",
                                               name=f"ctx{k}")
                                    for k in range(DT)]
                            for k in range(DT):
                                nc.sync.dma_start(
                                    CTX3[k][:],
                                    ag_out[128 * k:128 * (k + 1)][:,
                                                                  bass.ds(off,
                                                                          CH)])
                        XO = [p_xo.tile([128, D], F32, tag=f"xo{m}",
                                        name=f"xo{m}")
                              for m in range(MT)]
                        for m in range(MT):
                            nc.scalar.dma_start(
                                XO[m][:],
                                xown_d[ci * CH + 128 * m:
                                       ci * CH + 128 * (m + 1), :])
                        HM = [p_h.tile([128, D], F32, tag=f"h{m}", name=f"h{m}")
                              for m in range(MT)]
                        HT = [p_hT.tile([128, CH], mm_dt, tag=f"ht{k}",
                                        name=f"ht{k}")
                              for k in range(DT)]
                        # wo per m-pair so LN1/transposes overlap later pairs;
                        # uses the ps_mm pool so chunk B's wo does not contend
                        # with chunk A's ff2 PSUM slots.
                        for mp in range(0, MT, 2):
                            mlist = [q for q in (mp, mp + 1) if q < MT]
                            for nw in range(NWC):
                                WPS = {m: ps_mm.tile([128, WN], F32, tag="mm",
                                                     name="wps")
                                       for m in mlist}
                                for k in range(DT):
                                    woc = p_wos.tile([128, WN], mm_dt, tag="woc",
                                                     name="woc")
                                    nc.sync.dma_start(
                                        woc[:],
                                        woT_d[128 * k:128 * (k + 1),
                                              WN * nw:WN * (nw + 1)])
                                    for m in mlist:
                                        nc.tensor.matmul(
                                            WPS[m][:],
                                            CTX3[k][:, 128 * m:128 * (m + 1)],
                                            woc[:],
                                            start=(k == 0), stop=(k == DT - 1),
                                        )
                                sl = slice(WN * nw, WN * (nw + 1))
                                for m in mlist:
                                    nc.vector.tensor_add(XO[m][:, sl], WPS[m][:],
                                                         XO[m][:, sl])
                                    nc.vector.tensor_add(XO[m][:, sl],
                                                         XO[m][:, sl],
                                                         bor[:, sl])
                            for m in mlist:
                                layer_norm(XO[m], HM[m], XO[m])
                                for dk in range(DT):
                                    tp = ps_tp.tile([128, 128], F32, tag="tp",
                                                    name="tp")
                                    nc.tensor.transpose(
                                        tp[:], HM[m][:, 128 * dk:128 * (dk + 1)],
                                        ident[:])
                                    nc.vector.tensor_copy(
                                        HT[dk][:, 128 * m:128 * (m + 1)], tp[:])
                        UT = [p_uT.tile([128, CH], mm_dt, tag=f"u{i}",
                                        name=f"u{i}")
                              for i in range(FT)]
                        for i in range(FT):
                            w1c = p_w1s.tile([128, DT * 128], mm_dt, tag="w1c",
                                             name="w1c")
                            nc.sync.dma_start(
                                w1c[:].rearrange("p (k f) -> p k f", f=128),
                                w1p_d[:].rearrange("p (k f) -> p k f", k=DT)[
                                    :, :, 128 * i:128 * (i + 1)],
                            )
                            ps = ps_mm.tile([128, CH], F32, tag="mm", name="mm")
                            for k in range(DT):
                                nc.tensor.matmul(
                                    ps[:], w1c[:, 128 * k:128 * (k + 1)],
                                    HT[k][:],
                                    start=(k == 0), stop=(k == DT - 1),
                                )
                            nc.scalar.activation(UT[i][:], ps[:], AF.Relu,
                                                 bias=B1C[i][:])
                        ACC = [p_acc.tile([128, D], F32, tag=f"acc{m}",
                                          name=f"acc{m}")
                               for m in range(MT)]
                        for nw in range(NWC):
                            PS2 = [ps_f2.tile([128, WN], F32, tag=f"f2_{m}",
                                              name=f"f2_{m}")
                                   for m in range(MT)]
                            for k in range(FT):
                                w2c = p_w2s.tile([128, WN], mm_dt, tag="w2c",
                                                 name="w2c")
                                nc.sync.dma_start(
                                    w2c[:],
                                    w2T_d[128 * k:128 * (k + 1),
                                          WN * nw:WN * (nw + 1)])
                                for m in range(MT):
                                    nc.tensor.matmul(
                                        PS2[m][:],
                                        UT[k][:, 128 * m:128 * (m + 1)],
                                        w2c[:],
                                        start=(k == 0), stop=(k == FT - 1),
                                    )
                            sl = slice(WN * nw, WN * (nw + 1))
                            for m in range(MT):
                                nc.vector.tensor_add(ACC[m][:, sl], PS2[m][:],
                                                     b2r[:, sl])
                                nc.vector.tensor_add(ACC[m][:, sl],
                                                     ACC[m][:, sl],
                                                     HM[m][:, sl])
                        for m in range(MT):
                            o = p_work.tile([128, D], F32, tag="out", name="out")
                            layer_norm(ACC[m], o, ACC[m])
                            nc.sync.dma_start(
                                out_d[ci * CH + 128 * m:
                                      ci * CH + 128 * (m + 1), :],
                                o[:])
                p_ctx_cm.__exit__(None, None, None)

            for _rep in range(amp_reps):
                emit_layer()

    import time as _time
    _t0 = _time.monotonic()
    nc.compile()
    print(f"[build_nc] bacc/tile compile: {_time.monotonic() - _t0:.1f}s, "
          f"insts={sum(len(bb.instructions) for bb in nc.main_func.blocks)}")
    return nc


def shard_inputs(cfg, inputs):
    """Build the 8 per-core input maps from the full-problem inputs."""
    c = derive(cfg)
    T, D, FF = c["T"], c["D"], c["FF"]
    CH, DT, DSP, FT, DS, HALF, TCK = (
        c["CH"], c["DT"], c["DSP"], c["FT"], c["DS"], c["HALF"], c["TCK"])
    x = np.asarray(inputs["x"], np.float32)
    wq, wk, wv, wo = (np.asarray(inputs[k], np.float32)
                      for k in ("wq", "wk", "wv", "wo"))
    bq, bk, bv, bo = (np.asarray(inputs[k], np.float32)
                      for k in ("bq", "bk", "bv", "bo"))
    w1, b1, w2, b2 = (np.asarray(inputs[k], np.float32)
                      for k in ("w1", "b1", "w2", "b2"))

    ident = np.eye(128, dtype=np.float32)
    masks = np.zeros((TCK, 128, CH), np.float32)
    ii = np.arange(128)[:, None]
    jj = np.arange(CH)[None, :]
    for m in range(TCK):
        masks[m] = (jj >= ii + 128 * m).astype(np.float32)
    ones_bc = np.ones((1, 64), np.float32)

    w1T_packed = np.ascontiguousarray(
        w1.T.reshape(DT, 128, FF).transpose(1, 0, 2).reshape(128, DT * FF))
    b1c = np.ascontiguousarray(b1.reshape(FT, 128, 1))
    w2T = np.ascontiguousarray(w2.T)
    b2_rep = np.tile(b2[None, :], (128, 1))
    bo_rep = np.tile(bo[None, :], (128, 1))
    woT = np.ascontiguousarray(wo.T)

    in_maps = []
    for core in range(N_CORES):
        b, r = core // 2, core % 2
        hsl = slice(r * DS, (r + 1) * DS)
        in_maps.append({
            "xT": np.ascontiguousarray(x[b].T),
            "x_own": np.ascontiguousarray(x[b, r * HALF:(r + 1) * HALF]),
            "wqT": np.ascontiguousarray(wq[hsl].T),
            "wkT": np.ascontiguousarray(wk[hsl].T),
            "wvT": np.ascontiguousarray(wv[hsl].T),
            "bq8": np.ascontiguousarray(
                (bq[hsl] / np.sqrt(c["DK"])).reshape(DSP, 128, 1)),
            "bk": np.ascontiguousarray(bk[hsl].reshape(DSP, 128, 1)),
            "bv_rep": np.tile(bv[hsl][None, :], (128, 1)),
            "woT": woT,
            "bo_rep": bo_rep,
            "w1T_packed": w1T_packed,
            "b1c": b1c,
            "w2T": w2T,
            "b2_rep": b2_rep,
            "ident": ident,
            "masks": masks,
            "ones_bc": ones_bc,
            "coloff": np.array([[CH * r]], np.uint32),
        })
    return in_maps


def gather_outputs(cfg, results):
    c = derive(cfg)
    B, T, D, HALF = c["B"], c["T"], c["D"], c["HALF"]
    out = np.empty((B, T, D), np.float32)
    for core in range(N_CORES):
        b, r = core // 2, core % 2
        out[b, r * HALF:(r + 1) * HALF] = results[core]["out_own"]
    return out


_NC_CACHE = {}


def get_nc(cfg_key=None):
    cfg = FULL_CFG if cfg_key is None else dict(cfg_key)
    key = tuple(sorted(cfg.items()))
    if key not in _NC_CACHE:
        _NC_CACHE[key] = build_nc(cfg)
    return _NC_CACHE[key]


def run(cfg, inputs):
    nc = get_nc(tuple(sorted(cfg.items())))
    in_maps = shard_inputs(cfg, inputs)
    res = run_bass_kernel_spmd(nc, in_maps, core_ids=list(range(N_CORES)))
    return gather_outputs(cfg, res.results)


def kernel(**inputs) -> np.ndarray:
    """Full decoder layer: accepts the complete inputs, returns [4,2048,1024]."""
    return run(FULL_CFG, inputs)



# revision 32
# speedup vs baseline: 1.3055x; 1.3055x over previous
"""Trainium2 Bass kernel for nn_DecoderLayer (dense transformer decoder layer).

Distribution over 8 NeuronCores: core c = 2*b + r handles batch b (of 4) with
tensor-parallel rank r (of 2).
  - QKV + attention: rank r computes heads [8r, 8r+8) for ALL 2048 tokens of
    its batch. Q/K are produced head-transposed ([d_k, T]); V natural, with a
    ones-column appended per head so the PV matmul also yields the softmax
    denominator. Causality is exploited block-wise (upper blocks skipped,
    diagonal blocks masked after exp).
  - Two pairwise AllGathers (1 MB each per rank, bf16) re-shard the attention
    context from head-split to token-split, overlapped with compute.
  - wo projection, LN1, FFN, LN2 run fully local on the rank's own
    1024-token half. Rank-specific columns of the AllGather outputs are read
    via a register-driven dynamic DMA offset so one SPMD program serves both
    ranks.
Matmul operands are bf16 (psum accumulation fp32); layernorm statistics,
biases and residuals stay fp32. The host only reshapes/transposes/casts.
"""

import contextlib

import numpy as np
import ml_dtypes

import concourse.bass as bass
import concourse.bacc as bacc
import concourse.mybir as mybir
import concourse.tile as tile
from concourse.bass_utils import run_bass_kernel_spmd

F32 = mybir.dt.float32
F32R = mybir.dt.float32r
BF16 = mybir.dt.bfloat16
U32 = mybir.dt.uint32
AF = mybir.ActivationFunctionType
X_AXIS = mybir.AxisListType.X

N_CORES = 8
FULL_CFG = dict(T=2048, D=1024, H=16, DK=64, FF=4096, B=4)

# Optional phase-marker callback for profiling tools; no-op in production.
_phase_cb = None

# Temporary debug toggles for HW bisection (all-False is the production path).
import os as _os
_DBG_FLAT_EXP = _os.environ.get("KDBG_FLAT_EXP", "0") == "1"
_DBG_DVE_MEMSET = _os.environ.get("KDBG_DVE_MEMSET", "0") == "1"
_DBG_SIM_AG = _os.environ.get("KDBG_SIM_AG", "0") == "1"
_DBG_DUMP = _os.environ.get("KDBG_DUMP", "0") == "1"
_DBG_STOP = _os.environ.get("KDBG_STOP", "")
_DBG_P1LVL = int(_os.environ.get("KDBG_P1LVL", "9"))


class _StopEmit(Exception):
    pass


def _mark(nc, label):
    if _phase_cb is not None:
        _phase_cb(nc, label)


def derive(cfg):
    T, D, H, DK, FF, B = (cfg[k] for k in ("T", "D", "H", "DK", "FF", "B"))
    assert DK == 64 and H % 2 == 0 and T % 512 == 0 and D % 128 == 0
    HPC = H // 2
    DS = HPC * DK
    c = dict(cfg)
    c.update(
        HPC=HPC, DS=DS,
        CH=T // 4, KTN=T // 128, DT=D // 128, DSP=DS // 128,
        FT=FF // 128, HALF=T // 2,
    )
    c["TCK"] = c["CH"] // 128
    c["MT"] = c["CH"] // 128
    c["WN"] = min(512, D)
    c["NWC"] = D // c["WN"]
    assert c["CH"] <= 512
    return c


def build_nc(cfg, amp_reps=1, sim_mode=False, mm_dt=BF16):
    c = derive(cfg)
    T, D, FF = c["T"], c["D"], c["FF"]
    CH, KTN, DT, DSP, FT, MT = c["CH"], c["KTN"], c["DT"], c["DSP"], c["FT"], c["MT"]
    TCK, WN, NWC, DS, HALF, HPC = (
        c["TCK"], c["WN"], c["NWC"], c["DS"], c["HALF"], c["HPC"])
    scale = float(1.0 / np.sqrt(c["DK"]))

    nc = bacc.Bacc("TRN2", target_bir_lowering=False, num_devices=N_CORES)

    xT_d = nc.dram_tensor("xT", [D, T], mm_dt, kind="ExternalInput")
    xown_d = nc.dram_tensor("x_own", [HALF, D], F32, kind="ExternalInput")
    wqT_d = nc.dram_tensor("wqT", [D, DS], mm_dt, kind="ExternalInput")
    wkT_d = nc.dram_tensor("wkT", [D, DS], mm_dt, kind="ExternalInput")
    wvT_d = nc.dram_tensor("wvT", [D, DS], mm_dt, kind="ExternalInput")
    bq8_d = nc.dram_tensor("bq8", [DSP, 128, 1], F32, kind="ExternalInput")
    bk_d = nc.dram_tensor("bk", [DSP, 128, 1], F32, kind="ExternalInput")
    bvr_d = nc.dram_tensor("bv_rep", [128, DS], F32, kind="ExternalInput")
    woT_d = nc.dram_tensor("woT", [D, D], mm_dt, kind="ExternalInput")
    bor_d = nc.dram_tensor("bo_rep", [128, D], F32, kind="ExternalInput")
    w1p_d = nc.dram_tensor("w1T_packed", [128, FT * DT * 128], mm_dt,
                           kind="ExternalInput")
    b1c_d = nc.dram_tensor("b1c", [FT, 128, 1], F32, kind="ExternalInput")
    w2T_d = nc.dram_tensor("w2T", [FF, D], mm_dt, kind="ExternalInput")
    b2r_d = nc.dram_tensor("b2_rep", [128, D], F32, kind="ExternalInput")
    ident_d = nc.dram_tensor("ident", [128, 128], mm_dt, kind="ExternalInput")
    masks_d = nc.dram_tensor("masks2", [TCK, 128, 2 * CH], mm_dt,
                             kind="ExternalInput")
    coloff_d = nc.dram_tensor("coloff", [1, 1], U32, kind="ExternalInput")
    out_d = nc.dram_tensor("out_own", [HALF, D], F32, kind="ExternalOutput")
    dbg = {}
    if _DBG_DUMP:
        for nm, shape in (("dbg_xt", [128, T]), ("dbg_qt", [128, T]),
                          ("dbg_kt", [128, T]), ("dbg_va", [128, HPC * 65]),
                          ("dbg_ctx", [128, T]), ("dbg_ctx3", [128, CH]),
                          ("dbg_xo", [128, D]), ("dbg_hm", [128, D]),
                          ("dbg_ht", [128, CH]), ("dbg_ut", [128, CH]),
                          ("dbg_acc", [128, D]),
                          ("dbg_wr", [128, DS]), ("dbg_qt_early", [128, T])):
            dt = F32 if nm in ("dbg_xo", "dbg_acc") else mm_dt
            dbg[nm] = nc.dram_tensor(nm, shape, dt, kind="ExternalOutput")

    RG = [[0, 1], [2, 3], [4, 5], [6, 7]]

    with tile.TileContext(nc) as tc:
        with contextlib.ExitStack() as es:
            p_const = es.enter_context(tc.tile_pool(name="const", bufs=1))
            p_dram = es.enter_context(tc.tile_pool(name="dram", bufs=2, space="DRAM"))

            ident = p_const.tile([128, 128], mm_dt, tag="ident", name="ident")
            nc.sync.dma_start(ident[:], ident_d[:])

            offsb = p_const.tile([1, 1], U32, tag="offsb", name="offsb")
            nc.sync.dma_start(offsb[:], coloff_d[:])
            roff = nc.sync.alloc_register("roff")
            nc.sync.reg_load(roff, offsb[0:1, 0:1])
            off = nc.sync.snap(roff, min_val=0, max_val=CH)

            def emit_allgather(ag_i, ag_o):
                if sim_mode or _DBG_SIM_AG:
                    nc.scalar.dma_start(ag_o[0:DS, :], ag_i[:])
                    nc.scalar.dma_start(ag_o[DS:2 * DS, :], ag_i[:])
                else:
                    nc.gpsimd.collective_compute(
                        "AllGather", mybir.AluOpType.bypass,
                        replica_groups=RG,
                        ins=[ag_i.opt()], outs=[ag_o.opt()],
                    )

            def emit_layer():
                try:
                    _emit_layer_inner()
                except _StopEmit:
                    pass

            def _emit_layer_inner():
                # ======== Phases 1+2: QKV projections and attention ========
                p_ctx_cm = tc.tile_pool(name="ctx3", bufs=1)
                p_ctx = p_ctx_cm.__enter__()
                try:
                    CTX3A, ag1_out, ag2_out = _phases12(p_ctx)
                except _StopEmit:
                    p_ctx_cm.__exit__(None, None, None)
                    raise
                _phase3(p_ctx, CTX3A, ag1_out, ag2_out)
                p_ctx_cm.__exit__(None, None, None)

            def _phases12(p_ctx):
                ag1_in = p_dram.tile([DS, 2 * CH], mm_dt, tag="ag1i",
                                     name="ag1i")
                ag1_out = p_dram.tile([2 * DS, 2 * CH], mm_dt, tag="ag1o",
                                      name="ag1o")
                ag2_in = p_dram.tile([DS, 2 * CH], mm_dt, tag="ag2i",
                                     name="ag2i")
                ag2_out = p_dram.tile([2 * DS, 2 * CH], mm_dt, tag="ag2o",
                                      name="ag2o")
                with (
                    tc.tile_pool(name="qt", bufs=1) as p_qt,
                    tc.tile_pool(name="kt", bufs=1) as p_kt,
                    tc.tile_pool(name="vaug", bufs=1) as p_va,
                ):
                    QT = [p_qt.tile([128, T], mm_dt, tag=f"q{p}", name=f"q{p}")
                          for p in range(DSP)]
                    KT = [p_kt.tile([128, T], mm_dt, tag=f"k{p}", name=f"k{p}")
                          for p in range(DSP)]
                    VA = [p_va.tile([128, HPC * 65], mm_dt, tag=f"v{i}", name=f"v{i}")
                          for i in range(KTN)]

                    _mark(nc, 'qkv')
                    # ---- Phase 1: QKV ----
                    with (
                        tc.tile_pool(name="xT", bufs=1) as p_xT,
                        tc.tile_pool(name="wqk", bufs=2) as p_wqk,
                        tc.tile_pool(name="wv", bufs=1) as p_wv,
                        tc.tile_pool(name="bias1", bufs=1) as p_b1,
                        tc.tile_pool(name="ps_qk", bufs=1, space="PSUM") as ps_qk,
                        tc.tile_pool(name="ps_v", bufs=1, space="PSUM") as ps_v,
                    ):
                        XT = [p_xT.tile([128, T], mm_dt, tag=f"x{k}", name=f"x{k}")
                              for k in range(DT)]
                        for k in range(DT):
                            nc.sync.dma_start(XT[k][:],
                                              xT_d[128 * k:128 * (k + 1), :])
                        if _DBG_DUMP:
                            nc.sync.dma_start(dbg["dbg_xt"][:], XT[0][:])
                        bvr = p_b1.tile([128, DS], F32, tag="bvr", name="bvr")
                        NQ = T // CH
                        _wlist = [(wqT_d, bq8_d, QT, scale, "bq"),
                                  (wkT_d, bk_d, KT, 1.0, "bk")]
                        if _DBG_P1LVL < 2:
                            _wlist = _wlist[:1]
                        for (w_d, b_d, dst, sc, bt) in _wlist:
                            BB = [p_b1.tile([128, 1], F32, tag=f"{bt}{p}",
                                            name=f"{bt}{p}")
                                  for p in range(DSP)]
                            for p in range(DSP):
                                nc.sync.dma_start(BB[p][:], b_d[p])
                            WR = [p_wqk.tile([128, DS], mm_dt, tag=f"wr{k}",
                                             name=f"wr{k}")
                                  for k in range(DT)]
                            for k in range(DT):
                                nc.sync.dma_start(
                                    WR[k][:], w_d[128 * k:128 * (k + 1), :])
                            if _DBG_DUMP and bt == "bq":
                                nc.sync.dma_start(dbg["dbg_wr"][:], WR[0][:])
                            for p in range(DSP if _DBG_P1LVL >= 1 else 0):
                                PSN = {
                                    n: ps_qk.tile([128, CH], F32, tag=f"qk{n}",
                                                  name="psn")
                                    for n in range(NQ)
                                }
                                for k in range(DT):
                                    for n in range(NQ):
                                        nc.tensor.matmul(
                                            PSN[n][:],
                                            WR[k][:, 128 * p:128 * (p + 1)],
                                            XT[k][:, CH * n:CH * (n + 1)],
                                            start=(k == 0),
                                            stop=(k == DT - 1),
                                        )
                                for n in range(NQ):
                                    nc.scalar.activation(
                                        dst[p][:, CH * n:CH * (n + 1)],
                                        PSN[n][:],
                                        AF.Identity, bias=BB[p][:], scale=sc,
                                    )
                                if _DBG_DUMP and bt == "bq" and p == 0:
                                    nc.sync.dma_start(dbg["dbg_qt_early"][:],
                                                      dst[0][:])
                        _mark(nc, 'vproj')
                        if _DBG_P1LVL < 3:
                            if _DBG_DUMP:
                                nc.sync.dma_start(dbg["dbg_qt"][:], QT[0][:])
                            raise _StopEmit()
                        nc.sync.dma_start(bvr[:], bvr_d[:])
                        WV = [p_wv.tile([128, DS], mm_dt, tag=f"wv{k}",
                                        name=f"wv{k}")
                              for k in range(DT)]
                        for k in range(DT):
                            nc.sync.dma_start(WV[k][:],
                                              wvT_d[128 * k:128 * (k + 1), :])
                        for i in range(KTN):
                            ps = ps_v.tile([128, DS], F32, tag=f"v{i % 4}",
                                           name="psv")
                            for k in range(DT):
                                nc.tensor.matmul(
                                    ps[:], XT[k][:, 128 * i:128 * (i + 1)],
                                    WV[k][:],
                                    start=(k == 0), stop=(k == DT - 1),
                                )
                            va3 = VA[i][:].rearrange("p (h e) -> p h e", e=65)
                            nc.vector.tensor_add(
                                va3[:, :, 0:64],
                                ps[:].rearrange("p (h e) -> p h e", e=64),
                                bvr[:].rearrange("p (h e) -> p h e", e=64),
                            )
                            for h in range(HPC):
                                if _DBG_DVE_MEMSET:
                                    nc.vector.memset(
                                        VA[i][:, 65 * h + 64:65 * h + 65], 1.0)
                                else:
                                    nc.gpsimd.memset(
                                        VA[i][:, 65 * h + 64:65 * h + 65], 1.0)
                        if _DBG_DUMP:
                            nc.sync.dma_start(dbg["dbg_qt"][:], QT[0][:])
                            nc.sync.dma_start(dbg["dbg_kt"][:], KT[0][:])
                            nc.sync.dma_start(dbg["dbg_va"][:], VA[0][:])

                    if _DBG_STOP == "qkv":
                        raise _StopEmit()

                    _mark(nc, 'attn')
                    # ---- Phase 2: attention ----
                    with (
                        tc.tile_pool(name="ctxT", bufs=1) as p_ctxT,
                        tc.tile_pool(name="mask", bufs=1) as p_mask,
                        tc.tile_pool(name="exp", bufs=3) as p_exp,
                        tc.tile_pool(name="sm", bufs=2) as p_sm,
                        tc.tile_pool(name="ps_s", bufs=2, space="PSUM") as ps_s,
                        tc.tile_pool(name="ps_pv", bufs=2, space="PSUM") as ps_pv,
                    ):
                        CTX = [p_ctxT.tile([128, T], mm_dt, tag=f"c{p}",
                                           name=f"c{p}")
                               for p in range(DSP)]
                        MSK = [p_mask.tile([128, 2 * CH], mm_dt, tag=f"m{m}",
                                           name=f"m{m}")
                               for m in range(TCK)]
                        for m in range(TCK):
                            nc.sync.dma_start(MSK[m][:], masks_d[m])

                        def emit_norm(qc, p, pv):
                            rec = p_sm.tile([1, 2 * CH], F32, tag="rec",
                                            name="rec")
                            with nc.allow_low_precision(
                                    reason="denominator reciprocal"):
                                nc.vector.reciprocal(rec[:], pv[64:65, :])
                            recb = p_sm.tile([64, 2 * CH], F32, tag="recb",
                                             name="recb")
                            nc.gpsimd.partition_broadcast(recb[:], rec[:])
                            for hi in range(2):
                                hsl = slice(CH * hi, CH * (hi + 1))
                                nc.vector.tensor_mul(
                                    CTX[p][64 * hi:64 * (hi + 1),
                                           CH * qc:CH * (qc + 1)],
                                    pv[0:64, hsl], recb[:, hsl],
                                )

                        def emit_scores(qc, p, kt):
                            m = kt - qc * TCK
                            c0 = 128 * m if m > 0 else 0
                            s2 = ps_s.tile([128, 2 * CH], F32, tag="s2",
                                           name="s2")
                            nc.tensor.matmul(
                                s2[:, c0:CH],
                                KT[p][0:64, 128 * kt:128 * (kt + 1)],
                                QT[p][0:64, CH * qc + c0:CH * (qc + 1)],
                                start=True, stop=True,
                            )
                            nc.tensor.matmul(
                                s2[:, CH + c0:2 * CH],
                                KT[p][64:128, 128 * kt:128 * (kt + 1)],
                                QT[p][64:128, CH * qc + c0:CH * (qc + 1)],
                                start=True, stop=True,
                                tile_position=(64, 0),
                            )
                            return s2

                        def stage_ag1():
                            for p in range(DSP):
                                nc.scalar.dma_start(
                                    ag1_in[128 * p:128 * (p + 1), 0:CH],
                                    CTX[p][:, 0:CH])
                                nc.scalar.dma_start(
                                    ag1_in[128 * p:128 * (p + 1), CH:2 * CH],
                                    CTX[p][:, 2 * CH:3 * CH])
                            emit_allgather(ag1_in, ag1_out)

                        all_kts = [
                            (qc, p, kt, (qc + 1) * TCK)
                            for qc in (0, 2, 1, 3)
                            for p in range(DSP)
                            for kt in range((qc + 1) * TCK)
                        ]
                        pending = None
                        pv = None
                        s2 = emit_scores(*all_kts[0][:3])
                        for idx, (qc, p, kt, nkt) in enumerate(all_kts):
                            m = kt - qc * TCK
                            c0 = 128 * m if m > 0 else 0
                            cs = slice(c0, CH)
                            cs2 = slice(CH + c0, 2 * CH)
                            e2 = p_exp.tile([128, 2 * CH], mm_dt,
                                            tag="e2", name="e2")
                            if _DBG_FLAT_EXP:
                                nc.scalar.activation(e2[:, cs], s2[:, cs],
                                                     AF.Exp)
                                nc.scalar.activation(e2[:, cs2], s2[:, cs2],
                                                     AF.Exp)
                                if m >= 0:
                                    nc.vector.tensor_mul(
                                        e2[:, cs], e2[:, cs], MSK[m][:, cs])
                                    nc.vector.tensor_mul(
                                        e2[:, cs2], e2[:, cs2], MSK[m][:, cs2])
                            else:
                                s3 = s2[:].rearrange("p (u ch) -> p u ch", u=2)
                                e3 = e2[:].rearrange("p (u ch) -> p u ch", u=2)
                                nc.scalar.activation(e3[:, :, cs],
                                                     s3[:, :, cs], AF.Exp)
                                if m >= 0:
                                    m3 = MSK[m][:].rearrange(
                                        "p (u ch) -> p u ch", u=2)
                                    nc.vector.tensor_mul(
                                        e3[:, :, cs], e3[:, :, cs],
                                        m3[:, :, cs])
                            if idx + 1 < len(all_kts):
                                s2_next = emit_scores(*all_kts[idx + 1][:3])
                            else:
                                s2_next = None
                            if kt == 0:
                                pv = ps_pv.tile([65, 2 * CH], F32, tag="pv",
                                                name="pv")
                            nc.tensor.matmul(
                                pv[:, cs],
                                VA[kt][:, 130 * p:130 * p + 65],
                                e2[:, cs],
                                start=(kt == 0), stop=(kt == nkt - 1),
                            )
                            nc.tensor.matmul(
                                pv[:, cs2],
                                VA[kt][:, 130 * p + 65:130 * p + 130],
                                e2[:, cs2],
                                start=(kt == 0), stop=(kt == nkt - 1),
                            )
                            if kt == 1 and pending is not None:
                                do_ag1 = pending[:2] == (2, DSP - 1)
                                emit_norm(*pending)
                                pending = None
                                if do_ag1:
                                    stage_ag1()
                            if kt == nkt - 1:
                                pending = (qc, p, pv)
                            s2 = s2_next
                        if pending is not None:
                            emit_norm(*pending)
                            pending = None
                        if _DBG_DUMP:
                            nc.sync.dma_start(dbg["dbg_ctx"][:], CTX[0][:])
                        # chunk-A context prefetch: only waits on AG#1
                        CTX3A = [p_ctx.tile([128, CH], mm_dt, tag=f"ctx{k}",
                                            name=f"ctx{k}")
                                 for k in range(DT)]
                        for k in range(DT):
                            nc.sync.dma_start(
                                CTX3A[k][:],
                                ag1_out[128 * k:128 * (k + 1)][:,
                                                               bass.ds(off, CH)])
                        if _DBG_DUMP:
                            nc.sync.dma_start(dbg["dbg_ctx3"][:], CTX3A[0][:])
                        # AG#2 staging (waits end of attention)
                        for p in range(DSP):
                            nc.scalar.dma_start(
                                ag2_in[128 * p:128 * (p + 1), 0:CH],
                                CTX[p][:, CH:2 * CH])
                            nc.scalar.dma_start(
                                ag2_in[128 * p:128 * (p + 1), CH:2 * CH],
                                CTX[p][:, 3 * CH:4 * CH])
                        emit_allgather(ag2_in, ag2_out)

                return CTX3A, ag1_out, ag2_out

            def _phase3(p_ctx, CTX3A, ag1_out, ag2_out):
                _mark(nc, 'phase3')
                # ======== Phase 3: wo + LN1 + FFN + LN2, per token chunk ========
                with (
                    tc.tile_pool(name="rep", bufs=1) as p_rep,
                    tc.tile_pool(name="xo", bufs=1) as p_xo,
                    tc.tile_pool(name="hh", bufs=1) as p_h,
                    tc.tile_pool(name="hT", bufs=1) as p_hT,
                    tc.tile_pool(name="uT", bufs=1) as p_uT,
                    tc.tile_pool(name="acc", bufs=1) as p_acc,
                    tc.tile_pool(name="wk", bufs=2) as p_work,
                    tc.tile_pool(name="wos", bufs=4) as p_wos,
                    tc.tile_pool(name="w1s", bufs=3) as p_w1s,
                    tc.tile_pool(name="w2s", bufs=6) as p_w2s,
                    tc.tile_pool(name="st", bufs=4) as p_st,
                    tc.tile_pool(name="ps_mm", bufs=2, space="PSUM") as ps_mm,
                    tc.tile_pool(name="ps_f2", bufs=1, space="PSUM") as ps_f2,
                ):
                    bor = p_rep.tile([128, D], F32, tag="bor", name="bor")
                    nc.scalar.dma_start(bor[:], bor_d[:])
                    b2r = p_rep.tile([128, D], F32, tag="b2r", name="b2r")
                    nc.scalar.dma_start(b2r[:], b2r_d[:])
                    B1C = [p_rep.tile([128, 1], F32, tag=f"b1c{i}",
                                      name=f"b1c{i}")
                           for i in range(FT)]
                    for i in range(FT):
                        nc.gpsimd.dma_start(B1C[i][:], b1c_d[i])

                    def layer_norm(x_in, x_out, scratch):
                        s = p_st.tile([128, 1], F32, tag="s0", name="s0")
                        nc.vector.tensor_reduce(
                            s[:], x_in[:], axis=X_AXIS,
                            op=mybir.AluOpType.add, negate=True)
                        nm = p_st.tile([128, 1], F32, tag="s1", name="s1")
                        nc.scalar.mul(nm[:], s[:], 1.0 / D)
                        xc = p_work.tile([128, D], F32, tag="xc", name="xc")
                        nc.vector.tensor_scalar_add(xc[:], x_in[:], nm[:])
                        sq = p_st.tile([128, 1], F32, tag="s2", name="s2")
                        nc.scalar.activation(scratch[:], xc[:], AF.Square,
                                             accum_out=sq[:])
                        sd = p_st.tile([128, 1], F32, tag="s3", name="s3")
                        nc.scalar.activation(sd[:], sq[:], AF.Sqrt,
                                             scale=1.0 / (D - 1))
                        sde = p_st.tile([128, 1], F32, tag="s4", name="s4")
                        nc.vector.tensor_scalar_add(sde[:], sd[:], 1e-6)
                        rs = p_st.tile([128, 1], F32, tag="s5", name="s5")
                        nc.vector.reciprocal(rs[:], sde[:])
                        nc.vector.tensor_scalar_mul(x_out[:], xc[:], rs[:])

                    for ci, ag_out in ((0, ag1_out), (1, ag2_out)):
                        if ci == 0:
                            CTX3 = CTX3A
                        else:
                            CTX3 = [p_ctx.tile([128, CH], mm_dt, tag=f"ctx{k}",
                                               name=f"ctx{k}")
                                    for k in range(DT)]
                            for k in range(DT):
                                nc.sync.dma_start(
                                    CTX3[k][:],
                                    ag_out[128 * k:128 * (k + 1)][:,
                                                                  bass.ds(off,
                                                                          CH)])
                        XO = [p_xo.tile([128, D], F32, tag=f"xo{m}",
                                        name=f"xo{m}")
                              for m in range(MT)]
                        for m in range(MT):
                            nc.scalar.dma_start(
                                XO[m][:],
                                xown_d[ci * CH + 128 * m:
                                       ci * CH + 128 * (m + 1), :])
                        HM = [p_h.tile([128, D], mm_dt, tag=f"h{m}", name=f"h{m}")
                              for m in range(MT)]
                        HT = [p_hT.tile([128, CH], mm_dt, tag=f"ht{k}",
                                        name=f"ht{k}")
                              for k in range(DT)]
                        # wo per m-pair so LN1/transposes overlap later pairs
                        for mp in range(0, MT, 2):
                            mlist = [q for q in (mp, mp + 1) if q < MT]
                            for nw in range(NWC):
                                WPS = {m: ps_mm.tile([128, WN], F32, tag="mm",
                                                     name="wps")
                                       for m in mlist}
                                for k in range(DT):
                                    woc = p_wos.tile([128, WN], mm_dt, tag="woc",
                                                     name="woc")
                                    nc.sync.dma_start(
                                        woc[:],
                                        woT_d[128 * k:128 * (k + 1),
                                              WN * nw:WN * (nw + 1)])
                                    for m in mlist:
                                        nc.tensor.matmul(
                                            WPS[m][:],
                                            CTX3[k][:, 128 * m:128 * (m + 1)],
                                            woc[:],
                                            start=(k == 0), stop=(k == DT - 1),
                                        )
                                sl = slice(WN * nw, WN * (nw + 1))
                                for m in mlist:
                                    nc.vector.tensor_add(XO[m][:, sl], WPS[m][:],
                                                         XO[m][:, sl])
                                    nc.vector.tensor_add(XO[m][:, sl],
                                                         XO[m][:, sl],
                                                         bor[:, sl])
                            for m in mlist:
                                layer_norm(XO[m], HM[m], XO[m])
                                for dk in range(DT):
                                    tp = ps_mm.tile([128, 128], mm_dt, tag="mm",
                                                    name="tp")
                                    nc.tensor.transpose(
                                        tp[:], HM[m][:, 128 * dk:128 * (dk + 1)],
                                        ident[:])
                                    nc.vector.tensor_copy(
                                        HT[dk][:, 128 * m:128 * (m + 1)], tp[:])
                        if _DBG_DUMP and ci == 0:
                            nc.sync.dma_start(dbg["dbg_xo"][:], XO[0][:])
                            nc.sync.dma_start(dbg["dbg_hm"][:], HM[0][:])
                            nc.sync.dma_start(dbg["dbg_ht"][:], HT[0][:])
                        _mark(nc, 'ffn1')
                        UT = [p_uT.tile([128, CH], mm_dt, tag=f"u{i}",
                                        name=f"u{i}")
                              for i in range(FT)]
                        for i in range(FT):
                            w1c = p_w1s.tile([128, DT * 128], mm_dt, tag="w1c",
                                             name="w1c")
                            nc.sync.dma_start(
                                w1c[:],
                                w1p_d[:, DT * 128 * i:DT * 128 * (i + 1)])
                            ps = ps_mm.tile([128, CH], F32, tag="mm", name="mm")
                            for k in range(DT):
                                nc.tensor.matmul(
                                    ps[:], w1c[:, 128 * k:128 * (k + 1)],
                                    HT[k][:],
                                    start=(k == 0), stop=(k == DT - 1),
                                )
                            nc.vector.tensor_scalar(
                                UT[i][:], ps[:], B1C[i][:], 0.0,
                                op0=mybir.AluOpType.add,
                                op1=mybir.AluOpType.max)
                        _mark(nc, 'ffn2')
                        ACC = [p_acc.tile([128, D], F32, tag=f"acc{m}",
                                          name=f"acc{m}")
                               for m in range(MT)]
                        for nw in range(NWC):
                            PS2 = [ps_f2.tile([128, WN], F32, tag=f"f2_{m}",
                                              name=f"f2_{m}")
                                   for m in range(MT)]
                            for k in range(FT):
                                w2c = p_w2s.tile([128, WN], mm_dt, tag="w2c",
                                                 name="w2c")
                                nc.sync.dma_start(
                                    w2c[:],
                                    w2T_d[128 * k:128 * (k + 1),
                                          WN * nw:WN * (nw + 1)])
                                for m in range(MT):
                                    nc.tensor.matmul(
                                        PS2[m][:],
                                        UT[k][:, 128 * m:128 * (m + 1)],
                                        w2c[:],
                                        start=(k == 0), stop=(k == FT - 1),
                                    )
                            sl = slice(WN * nw, WN * (nw + 1))
                            for m in range(MT):
                                nc.vector.tensor_add(ACC[m][:, sl], PS2[m][:],
                                                     b2r[:, sl])
                                nc.vector.tensor_add(ACC[m][:, sl],
                                                     ACC[m][:, sl],
                                                     HM[m][:, sl])
                        if _DBG_DUMP and ci == 0:
                            nc.sync.dma_start(dbg["dbg_ut"][:], UT[0][:])
                            nc.sync.dma_start(dbg["dbg_acc"][:], ACC[0][:])
                        for m in range(MT):
                            o = p_work.tile([128, D], F32, tag="out", name="out")
                            layer_norm(ACC[m], o, ACC[m])
                            nc.gpsimd.dma_start(
                                out_d[ci * CH + 128 * m:
                                      ci * CH + 128 * (m + 1), :],
                                o[:])

            for _rep in range(amp_reps):
                emit_layer()

    import time as _time
    _t0 = _time.monotonic()
    nc.compile()
    print(f"[build_nc] bacc/tile compile: {_time.monotonic() - _t0:.1f}s, "
          f"insts={sum(len(bb.instructions) for bb in nc.main_func.blocks)}")
    return nc


def shard_inputs(cfg, inputs):
    """Build the 8 per-core input maps from the full-problem inputs."""
    c = derive(cfg)
    T, D, FF = c["T"], c["D"], c["FF"]
    CH, DT, DSP, FT, DS, HALF, TCK = (
        c["CH"], c["DT"], c["DSP"], c["FT"], c["DS"], c["HALF"], c["TCK"])
    bf = ml_dtypes.bfloat16
    x = np.asarray(inputs["x"], np.float32)
    wq, wk, wv, wo = (np.asarray(inputs[k], np.float32)
                      for k in ("wq", "wk", "wv", "wo"))
    bq, bk, bv, bo = (np.asarray(inputs[k], np.float32)
                      for k in ("bq", "bk", "bv", "bo"))
    w1, b1, w2, b2 = (np.asarray(inputs[k], np.float32)
                      for k in ("w1", "b1", "w2", "b2"))

    ident = np.eye(128, dtype=np.float32).astype(bf)
    masks = np.zeros((TCK, 128, CH), np.float32)
    ii = np.arange(128)[:, None]
    jj = np.arange(CH)[None, :]
    for m in range(TCK):
        masks[m] = (jj >= ii + 128 * m).astype(np.float32)
    masks2 = np.concatenate([masks, masks], axis=2).astype(bf)

    # w1 packed so each FFN1 i-slice [128, DT*128] is a contiguous DRAM read:
    # w1P[p, i, k, f] = w1[128*i + f, 128*k + p]
    w1P = np.ascontiguousarray(
        w1.reshape(FT, 128, DT, 128).transpose(3, 0, 2, 1).reshape(
            128, FT * DT * 128)).astype(bf)
    b1c = np.ascontiguousarray(b1.reshape(FT, 128, 1))
    w2T = np.ascontiguousarray(w2.T).astype(bf)
    b2_rep = np.tile(b2[None, :], (128, 1))
    bo_rep = np.tile(bo[None, :], (128, 1))
    woT = np.ascontiguousarray(wo.T).astype(bf)

    in_maps = []
    for core in range(N_CORES):
        b, r = core // 2, core % 2
        hsl = slice(r * DS, (r + 1) * DS)
        in_maps.append({
            "xT": np.ascontiguousarray(x[b].T).astype(bf),
            "x_own": np.ascontiguousarray(x[b, r * HALF:(r + 1) * HALF]),
            "wqT": np.ascontiguousarray(wq[hsl].T).astype(bf),
            "wkT": np.ascontiguousarray(wk[hsl].T).astype(bf),
            "wvT": np.ascontiguousarray(wv[hsl].T).astype(bf),
            "bq8": np.ascontiguousarray(
                (bq[hsl] / np.sqrt(c["DK"])).reshape(DSP, 128, 1)),
            "bk": np.ascontiguousarray(bk[hsl].reshape(DSP, 128, 1)),
            "bv_rep": np.tile(bv[hsl][None, :], (128, 1)),
            "woT": woT,
            "bo_rep": bo_rep,
            "w1T_packed": w1P,
            "b1c": b1c,
            "w2T": w2T,
            "b2_rep": b2_rep,
            "ident": ident,
            "masks2": masks2,
            "coloff": np.array([[CH * r]], np.uint32),
        })
    return in_maps


def gather_outputs(cfg, results):
    c = derive(cfg)
    B, T, D, HALF = c["B"], c["T"], c["D"], c["HALF"]
    out = np.empty((B, T, D), np.float32)
    for core in range(N_CORES):
        b, r = core // 2, core % 2
        out[b, r * HALF:(r + 1) * HALF] = results[core]["out_own"]
    return out


_NC_CACHE = {}


def get_nc(cfg_key=None):
    cfg = FULL_CFG if cfg_key is None else dict(cfg_key)
    key = tuple(sorted(cfg.items()))
    if key not in _NC_CACHE:
        _NC_CACHE[key] = build_nc(cfg)
    return _NC_CACHE[key]


def run(cfg, inputs):
    nc = get_nc(tuple(sorted(cfg.items())))
    in_maps = shard_inputs(cfg, inputs)
    res = run_bass_kernel_spmd(nc, in_maps, core_ids=list(range(N_CORES)))
    return gather_outputs(cfg, res.results)


def kernel(**inputs) -> np.ndarray:
    """Full decoder layer: accepts the complete inputs, returns [4,2048,1024]."""
    return run(FULL_CFG, inputs)
